# revision 1
# baseline (speedup 1.0000x reference)
"""GraphSAGE 2-layer mini-batch kernel for 8 Trainium2 NeuronCores.

Strategy: data-parallel over the batch (128 targets per core), x replicated.
The dominant cost is gathering ~36.6K random 512B rows of x per core.

Big gathers (nb1_self, nb1_nb: 35200 rows in 11 "chains" of 128 groups x 25)
use the Q7 dma_gather custom instruction, which needs int16 indices: indices
are bucket-sorted into 16 buckets of 32768 rows, one dma_gather per bucket,
spread over 4 SWDGE queues.  Bucketing permutes rows, so mean-aggregation is
done with data-dependent selection matrices built on-device (is_equal of
per-row local group ids against an iota, batched across tiles) feeding
single-pass float32r PE matmuls that accumulate transposed [feature, group]
sums directly in PSUM.  The 1/25 and 1/10 mean scalings are folded into
host-prescaled copies of W1/W2.

nodes + nb2 (1408 rows) are gathered in exact order with indirect DMA and
transposed on the PE.

Both SAGE layers run in transposed [feature, row] layout: W is the stationary
matmul operand, L2 norms use a ones-vector matmul for the cross-partition
reduction, a rank-1 matmul to broadcast the norm back across partitions, and
an elementwise divide.
"""
import sys

sys.path.insert(0, "/opt/trn_rl_repo")

import numpy as np

P = 128
D = 128
B = 1024
S1 = 25
S2 = 10
N_NODES = 500000
NCORES = 8
B_LOC = B // NCORES          # 128 targets per core
NCHAIN = 11                  # nb1_self + 10 nb1_nb chains
BUCKET_BITS = 15
BUCKET = 1 << BUCKET_BITS    # 32768 rows per bucket (int16 addressable)
NBUK = (N_NODES + BUCKET - 1) // BUCKET  # 16
NQ = 4                       # SWDGE queues
W_SEL = 2 * P                # sel window: 2 chains
K_SEL = 8                    # sel entries built per DVE op


def _prep_indices(nodes, nb2, nb1_self, nb1_nb):
    """Bucket-sort the big-gather indices per core; build device-side arrays
    and the (core-independent) per-entry metadata.

    Entry = (tile, base_chain): a 256-wide sel window covering chains
    {base, base+1}.  Tiles spanning more than 2 chains get several entries.
    """
    per_core = []
    for c in range(NCORES):
        sl = slice(c * B_LOC, (c + 1) * B_LOC)
        n1s = nb1_self[sl]              # [128, 25]
        n1n = nb1_nb[sl]                # [128, 10, 25]
        idx_chains = [n1s.reshape(-1).astype(np.int64)]
        grp_chains = [np.repeat(np.arange(B_LOC, dtype=np.int64), S1)]
        for j in range(S2):
            idx_chains.append(n1n[:, j, :].reshape(-1).astype(np.int64))
            grp_chains.append((j + 1) * B_LOC
                              + np.repeat(np.arange(B_LOC, dtype=np.int64), S1))
        all_idx = np.concatenate(idx_chains)   # [35200]
        all_grp = np.concatenate(grp_chains)
        bkt = all_idx >> BUCKET_BITS
        order = np.argsort(bkt, kind="stable")
        sidx, sgrp, sbkt = all_idx[order], all_grp[order], bkt[order]
        locs, grps = [], []
        for b in range(NBUK):
            m = sbkt == b
            locs.append((sidx[m] - (b << BUCKET_BITS)).astype(np.int64))
            grps.append(sgrp[m])
        per_core.append((locs, grps))

    # consistent per-bucket tile counts across cores (SPMD: one program)
    Cb = [max((len(per_core[c][0][b]) + P - 1) // P for c in range(NCORES))
          for b in range(NBUK)]
    Cb = [max(cb, 1) for cb in Cb]
    T_total = sum(Cb)
    S_total = T_total * P

    idx16_cores, grp_glob = [], []
    for c in range(NCORES):
        locs, grps = per_core[c]
        lidx = np.zeros(S_total, np.int64)
        lgrp = np.full(S_total, -1000.0, np.float64)
        off = 0
        for b in range(NBUK):
            n = len(locs[b])
            lidx[off:off + n] = locs[b]
            lgrp[off:off + n] = grps[b]
            off += Cb[b] * P
        wrapped_cols = []
        off = 0
        for b in range(NBUK):
            nb_pad = Cb[b] * P
            w = lidx[off:off + nb_pad].reshape(-1, 16).T.astype(np.int16)
            wrapped_cols.append(np.tile(w, (8, 1)))
            off += nb_pad
        idx16_cores.append(np.hstack(wrapped_cols))        # [128, S_total//16]
        grp_glob.append(lgrp.reshape(T_total, P).T)        # [128, T_total]

    # per-tile chain spans, unioned across cores so metadata is SPMD-safe
    tile_span = []
    for t in range(T_total):
        cmin, cmax = None, None
        for c in range(NCORES):
            g = grp_glob[c][:, t]
            v = g[g >= 0]
            if v.size:
                lo, hi = int(v.min()) // P, int(v.max()) // P
                cmin = lo if cmin is None else min(cmin, lo)
                cmax = hi if cmax is None else max(cmax, hi)
        tile_span.append((cmin, cmax))

    # entries: (tile, base_chain, chains) with 256-wide windows
    tile_base = np.cumsum([0] + Cb)
    entries_by_bucket = []
    for b in range(NBUK):
        ents = []
        for tl in range(Cb[b]):
            t = tile_base[b] + tl
            cmin, cmax = tile_span[t]
            if cmin is None:
                continue
            be = cmin
            while be <= cmax:
                chains = [ch for ch in (be, be + 1) if be <= ch <= cmax]
                ents.append((t, be, chains))
                be += 2
        entries_by_bucket.append(ents)

    # per-core grp data laid out per ENTRY, values local to the entry window
    E_total = sum(len(e) for e in entries_by_bucket)
    grp_ent_cores = []
    for c in range(NCORES):
        ge = np.full((P, E_total), -1000.0, np.float32)
        e = 0
        for b in range(NBUK):
            for (t, be, chains) in entries_by_bucket[b]:
                ge[:, e] = (grp_glob[c][:, t] - be * P).astype(np.float32)
                e += 1
        grp_ent_cores.append(ge)

    return dict(Cb=Cb, T_total=T_total, tile_base=tile_base,
                entries_by_bucket=entries_by_bucket, E_total=E_total,
                idx16_cores=idx16_cores, grp_ent_cores=grp_ent_cores)


def _build_program(meta, trace_sim=False, debug=False):
    import concourse.bacc as bacc_mod
    import concourse.bass as bass
    import concourse.tile as tile
    from concourse import mybir

    f32 = mybir.dt.float32
    f32r = mybir.dt.float32r
    Cb = meta["Cb"]
    T_total = meta["T_total"]
    tile_base = meta["tile_base"]
    entries_by_bucket = meta["entries_by_bucket"]
    E_total = meta["E_total"]

    nc = bacc_mod.Bacc(num_swdge_queues=NQ)

    x_d = nc.declare_dram_parameter("x", [N_NODES, D], f32, isOutput=False)
    w1a_d = nc.declare_dram_parameter("w1a", [D, D], f32, isOutput=False)
    w1b_d = nc.declare_dram_parameter("w1b", [D, D], f32, isOutput=False)
    w2a_d = nc.declare_dram_parameter("w2a", [D, D], f32, isOutput=False)
    w2b_d = nc.declare_dram_parameter("w2b", [D, D], f32, isOutput=False)
    b1_d = nc.declare_dram_parameter("b1v", [D, 1], f32, isOutput=False)
    b2_d = nc.declare_dram_parameter("b2v", [D, 1], f32, isOutput=False)
    ident_d = nc.declare_dram_parameter("ident", [P, P], f32, isOutput=False)
    ones_d = nc.declare_dram_parameter("onesm", [P, P], f32, isOutput=False)
    iota_d = nc.declare_dram_parameter("iota", [P, K_SEL * W_SEL], f32,
                                       isOutput=False)
    idx16_d = nc.declare_dram_parameter("idx16", [P, T_total * 8],
                                        mybir.dt.int16, isOutput=False)
    grp_d = nc.declare_dram_parameter("grp", [P, E_total], f32, isOutput=False)
    idx32_d = nc.declare_dram_parameter("idx32", [P, NCHAIN],
                                        mybir.dt.int32, isOutput=False)
    zt_d = nc.declare_dram_parameter("zt", [D, B_LOC], f32, isOutput=True)
    dbg_d = None
    if debug:
        dbg_d = nc.declare_dram_parameter("dbg", [D, (NCHAIN + 1) * P], f32,
                                          isOutput=True)

    with tile.TileContext(nc, trace_sim=trace_sim) as tc:
        with (
            tc.tile_pool(name="consts", bufs=1) as consts,
            tc.tile_pool(name="acts", bufs=1) as acts,
            tc.tile_pool(name="gbuf", bufs=6) as gpool,
            tc.tile_pool(name="lg", bufs=3) as lgpool,
            tc.tile_pool(name="selp", bufs=4) as selpool,
            tc.tile_pool(name="scratch", bufs=6) as scratch,
        ):
          with (
            tc.tile_pool(name="pagg", bufs=1, space="PSUM") as pagg,
            tc.tile_pool(name="ptr", bufs=2, space="PSUM") as ptr,
          ):
              # ---- gather-critical loads first ------------------------------
              idx16 = []
              for b in range(NBUK):
                  it = consts.tile([P, Cb[b] * 8], mybir.dt.int16,
                                   tag=f"idx16_{b}", name=f"idx16_{b}")
                  lo8, hi8 = tile_base[b] * 8, (tile_base[b] + Cb[b]) * 8
                  nc.sync.dma_start(out=it[:], in_=idx16_d[:, lo8:hi8])
                  idx16.append(it)
              grpc = consts.tile([P, E_total], f32, tag="grpc")
              iota = consts.tile([P, K_SEL * W_SEL], f32, tag="iota")
              nc.sync.dma_start(out=grpc[:], in_=grp_d[:])
              nc.sync.dma_start(out=iota[:], in_=iota_d[:])

              agg_ps = [pagg.tile([P, 4 * P], f32, tag=f"agg{k}", name=f"agg{k}")
                        for k in range(3)]

              def agg_slice(ch):
                  return agg_ps[ch // 4][:, (ch % 4) * P:(ch % 4 + 1) * P]

              # start=True resets a whole PSUM bank -> only the first matmul
              # touching each bank may set it (4 chains share a bank)
              def entry_mms(be, chains):
                  if len(chains) == 2 and be % 4 != 3:
                      return [(be, 2)]          # fused: two adjacent columns
                  return [(ch, 1) for ch in chains]

              pair_list = []
              for b in range(NBUK):
                  for (t, be, chains) in entries_by_bucket[b]:
                      for (ch, w) in entry_mms(be, chains):
                          pair_list.append((ch, w))
              first_pair, last_pair = {}, {}
              for i, (ch, w) in enumerate(pair_list):
                  banks = {ch // 4, (ch + w - 1) // 4}
                  for bank in banks:
                      if bank not in first_pair:
                          first_pair[bank] = i
                      last_pair[bank] = i

              # ---- dispatch big bucketed gathers ----------------------------
              gtiles = {}

              def emit_gather_part(b, c0, c1):
                  cb = Cb[b]
                  if b not in gtiles:
                      gtiles[b] = gpool.tile([P, cb * P], f32r, tag="gb",
                                             name=f"g{b}")
                  g = gtiles[b]
                  lo = b * BUCKET
                  hi = min(lo + BUCKET, N_NODES)
                  n = c1 - c0
                  g3 = g[:, c0 * D:c1 * D].rearrange("p (c e) -> p c e", c=n)
                  nc.gpsimd.dma_gather(
                      out_ap=g3,
                      in_ap=x_d[lo:hi, :].bitcast(f32r),
                      idxs_ap=idx16[b][:, c0 * 8:c1 * 8],
                      num_idxs=n * P,
                      num_idxs_reg=n * P,
                      elem_size=D,
                      single_packet=False,
                      queue_num=b % NQ,
                  )

              def dispatch_gather(b):
                  emit_gather_part(b, 0, Cb[b])

              # small head calls first so all 4 queues start streaming at once
              HEAD = 3
              for b in range(NQ):
                  emit_gather_part(b, 0, HEAD)
              for b in range(NQ):
                  emit_gather_part(b, HEAD, Cb[b])

              # ---- remaining const loads + little gathers -------------------
              w1a = consts.tile([D, D], f32r, tag="w1a")
              w1b = consts.tile([D, D], f32r, tag="w1b")
              w2a = consts.tile([D, D], f32r, tag="w2a")
              w2b = consts.tile([D, D], f32r, tag="w2b")
              b1t = consts.tile([D, 1], f32, tag="b1t")
              b2t = consts.tile([D, 1], f32, tag="b2t")
              ident = consts.tile([P, P], f32r, tag="ident")
              idx32 = consts.tile([P, NCHAIN], mybir.dt.int32, tag="idx32")
              ones = consts.tile([P, P], f32r, tag="ones")
              eps = consts.tile([P, 1], f32, tag="eps")
              nc.vector.memset(eps[:], 1e-30)
              for dst, srcd in ((w1a, w1a_d), (w1b, w1b_d), (w2a, w2a_d),
                                (w2b, w2b_d)):
                  nc.sync.dma_start(out=dst[:], in_=srcd[:].bitcast(f32r))
              for dst, srcd in ((b1t, b1_d), (b2t, b2_d), (idx32, idx32_d)):
                  nc.sync.dma_start(out=dst[:], in_=srcd[:])
              nc.sync.dma_start(out=ident[:], in_=ident_d[:].bitcast(f32r))
              nc.sync.dma_start(out=ones[:], in_=ones_d[:].bitcast(f32r))

              for b in range(NQ, 12):
                  dispatch_gather(b)

              # chain 0 = x[nodes]; chain 1+j = x[nb2[:, j]]
              self_sb = []
              for j in range(NCHAIN):
                  lt = lgpool.tile([P, D], f32r, tag="lt", name=f"lt{j}")
                  nc.gpsimd.indirect_dma_start(
                      out=lt[:], out_offset=None, in_=x_d[:].bitcast(f32r),
                      in_offset=bass.IndirectOffsetOnAxis(ap=idx32[:, j:j + 1],
                                                          axis=0),
                  )
                  ps = ptr.tile([P, P], f32, tag="pt", name=f"pt{j}")
                  nc.tensor.transpose(out=ps[:].bitcast(f32r), in_=lt[:],
                                      identity=ident[:])
                  st = acts.tile([D, B_LOC], f32r, tag=f"selfT{j}",
                                 name=f"selfT{j}")
                  nc.scalar.copy(out=st[:], in_=ps[:])
                  self_sb.append(st)

              for b in range(12, NBUK):
                  dispatch_gather(b)

              # ---- batched sel builds + f32r aggregation matmuls ------------
              pi = 0
              e_col = 0
              for b in range(NBUK):
                  ents = entries_by_bucket[b]
                  g = gtiles[b]
                  for c0 in range(0, len(ents), K_SEL):
                      chunk = ents[c0:c0 + K_SEL]
                      k = len(chunk)
                      sel = selpool.tile([P, K_SEL * W_SEL], f32r, tag="sel",
                                         name=f"sel{b}_{c0}")
                      nc.vector.tensor_tensor(
                          out=sel[:, :k * W_SEL].rearrange("p (k w) -> p k w",
                                                           k=k),
                          in0=grpc[:, e_col:e_col + k].broadcast_to(
                              [P, k, W_SEL]),
                          in1=iota[:, :k * W_SEL].rearrange("p (k w) -> p k w",
                                                            k=k),
                          op=mybir.AluOpType.is_equal,
                      )
                      for ke, (t, be, chains) in enumerate(chunk):
                          tl = t - tile_base[b]
                          gt = g[:, tl * D:(tl + 1) * D]
                          for (ch, w) in entry_mms(be, chains):
                              ps0 = agg_ps[ch // 4]
                              off = (ch % 4) * P
                              nc.tensor.matmul(
                                  out=ps0[:, off:off + w * P],
                                  lhsT=gt,
                                  rhs=sel[:, ke * W_SEL + (ch - be) * P:
                                          ke * W_SEL + (ch - be + w) * P],
                                  start=(first_pair[ch // 4] == pi),
                                  stop=(last_pair[ch // 4] == pi),
                                  skip_group_check=True,
                              )
                              pi += 1
                      e_col += k

              # ---- copy aggregated sums PSUM -> SBUF ------------------------
              agg_sb = []
              for ch in range(NCHAIN):
                  a = acts.tile([D, B_LOC], f32r, tag=f"aggT{ch}",
                                name=f"aggT{ch}")
                  nc.scalar.copy(out=a[:], in_=agg_slice(ch))
                  agg_sb.append(a)

              if debug:
                  for ch in range(NCHAIN):
                      nc.sync.dma_start(
                          out=dbg_d[:, ch * P:(ch + 1) * P].bitcast(f32r),
                          in_=agg_sb[ch][:])
                  nc.sync.dma_start(
                      out=dbg_d[:, NCHAIN * P:(NCHAIN + 1) * P].bitcast(f32r),
                      in_=self_sb[0][:])

          # ---- SAGE layer in transposed layout (agg PSUM banks now free) --
          with tc.tile_pool(name="psage", bufs=6, space="PSUM") as psage:
            h1n_all = acts.tile([P, S2 * P], f32r, tag="h1n_all")

            def sage_group(specs):
                """Stage-major emission of several independent SAGE heads so
                the engines pipeline across them."""
                n = len(specs)
                phs, hs, h2s, psss, nvs, pbcs, nrs = [], [], [], [], [], [], []
                for i, (rs, ra, wa, wb, bt, tagn, hn) in enumerate(specs):
                    ph = psage.tile([P, P], f32, tag="ps", name=f"ph_{tagn}")
                    nc.tensor.matmul(out=ph[:], lhsT=wa[:], rhs=rs,
                                     start=True, stop=False,
                                     skip_group_check=True)
                    nc.tensor.matmul(out=ph[:], lhsT=wb[:], rhs=ra,
                                     start=False, stop=True,
                                     skip_group_check=True)
                    phs.append(ph)
                for i, (rs, ra, wa, wb, bt, tagn, hn) in enumerate(specs):
                    h = scratch.tile([P, P], f32, tag="h", name=f"h_{tagn}")
                    nc.vector.tensor_scalar(out=h[:], in0=phs[i][:],
                                            scalar1=bt[:, :1], scalar2=0.0,
                                            op0=mybir.AluOpType.add,
                                            op1=mybir.AluOpType.max)
                    hs.append(h)
                for i, (rs, ra, wa, wb, bt, tagn, hn) in enumerate(specs):
                    h2 = scratch.tile([P, P], f32r, tag="h2", name=f"h2_{tagn}")
                    nc.scalar.square(out=h2[:], in_=hs[i][:])
                    h2s.append(h2)
                for i, (rs, ra, wa, wb, bt, tagn, hn) in enumerate(specs):
                    pss = psage.tile([P, P], f32, tag="ps", name=f"pss_{tagn}")
                    nc.tensor.matmul(out=pss[:1, :], lhsT=ones[:, :1],
                                     rhs=h2s[i][:], start=True, stop=True,
                                     skip_group_check=True)
                    psss.append(pss)
                for i, (rs, ra, wa, wb, bt, tagn, hn) in enumerate(specs):
                    nv = scratch.tile([P, P], f32r, tag="nv", name=f"nv_{tagn}")
                    nc.scalar.activation(
                        out=nv[:1, :], in_=psss[i][:1, :],
                        func=mybir.ActivationFunctionType.Sqrt,
                        bias=eps[:1, :1])
                    nvs.append(nv)
                for i, (rs, ra, wa, wb, bt, tagn, hn) in enumerate(specs):
                    pbc = psage.tile([P, P], f32, tag="ps", name=f"pbc_{tagn}")
                    nc.tensor.matmul(out=pbc[:], lhsT=ones[:1, :],
                                     rhs=nvs[i][:1, :], start=True, stop=True,
                                     skip_group_check=True)
                    pbcs.append(pbc)
                for i, (rs, ra, wa, wb, bt, tagn, hn) in enumerate(specs):
                    nr = scratch.tile([P, P], f32, tag="nr", name=f"nr_{tagn}")
                    nc.vector.reciprocal_approx_fast(out=nr[:], in_=pbcs[i][:])
                    nrs.append(nr)
                outs = []
                for i, (rs, ra, wa, wb, bt, tagn, hn) in enumerate(specs):
                    if hn is None:
                        hn = acts.tile([D, B_LOC], f32r, tag=tagn,
                                       name=tagn)[:]
                    nc.vector.tensor_tensor(out=hn, in0=hs[i][:], in1=nrs[i][:],
                                            op=mybir.AluOpType.mult)
                    outs.append(hn)
                return outs

            h1n_slice = lambda j: h1n_all[:, j * P:(j + 1) * P]
            specs = [(self_sb[0][:], agg_sb[0][:], w1a, w1b, b1t, "h1t", None)]
            specs += [(self_sb[1 + j][:], agg_sb[1 + j][:], w1a, w1b, b1t,
                       f"h1n{j}", h1n_slice(j)) for j in range(S2)]
            res0 = sage_group(specs[:4])
            h1t = res0[0]
            sage_group(specs[4:8])
            sage_group(specs[8:])

            a3 = acts.tile([D, B_LOC], f32r, tag="a3")
            with nc.allow_low_precision("f32r is 4-byte fp32 bits"):
                nc.vector.reduce_sum(
                    out=a3[:],
                    in_=h1n_all[:].rearrange("p (j r) -> p r j", j=S2),
                    axis=mybir.AxisListType.X,
                )

            zt = sage_group([(h1t, a3[:], w2a, w2b, b2t, "zt", None)])[0]
            nc.sync.dma_start(out=zt_d[:].bitcast(f32r), in_=zt)

    nc.finalize()
    return nc


def kernel(x, W1, b1, W2, b2, nodes, nb2, nb1_self, nb1_nb,
           _trace=False, _core_ids=None, _debug=False):
    x = np.ascontiguousarray(np.asarray(x, dtype=np.float32))
    W1 = np.asarray(W1, dtype=np.float32)
    W2 = np.asarray(W2, dtype=np.float32)
    b1 = np.asarray(b1, dtype=np.float32)
    b2 = np.asarray(b2, dtype=np.float32)
    nodes = np.asarray(nodes)
    nb2 = np.asarray(nb2)
    nb1_self = np.asarray(nb1_self)
    nb1_nb = np.asarray(nb1_nb)

    meta = _prep_indices(nodes, nb2, nb1_self, nb1_nb)
    nc = _build_program(meta, debug=_debug)

    # host-prescaled weights: the 1/25 and 1/10 means fold into W*b
    w1a = np.ascontiguousarray(W1[:D])
    w1b = np.ascontiguousarray(W1[D:] / S1)
    w2a = np.ascontiguousarray(W2[:D])
    w2b = np.ascontiguousarray(W2[D:] / S2)
    ident = np.eye(P, dtype=np.float32)
    iota = np.broadcast_to(
        np.tile(np.arange(W_SEL, dtype=np.float32), K_SEL), (P, K_SEL * W_SEL))
    iota = np.ascontiguousarray(iota)

    in_maps = []
    for c in range(NCORES):
        sl = slice(c * B_LOC, (c + 1) * B_LOC)
        idx32 = np.empty((P, NCHAIN), np.int32)
        idx32[:, 0] = nodes[sl]
        idx32[:, 1:] = nb2[sl]
        in_maps.append({
            "x": x,
            "w1a": w1a, "w1b": w1b, "w2a": w2a, "w2b": w2b,
            "b1v": b1.reshape(D, 1), "b2v": b2.reshape(D, 1),
            "ident": ident, "iota": iota,
            "onesm": np.ones((P, P), np.float32),
            "idx16": meta["idx16_cores"][c], "grp": meta["grp_ent_cores"][c],
            "idx32": idx32,
        })

    from concourse.bass_utils import run_bass_kernel_spmd

    core_ids = _core_ids if _core_ids is not None else list(range(NCORES))
    res = run_bass_kernel_spmd(nc, in_maps[:len(core_ids)], core_ids=core_ids,
                               trace=_trace)
    z = np.concatenate([res.results[c]["zt"].T for c in range(len(core_ids))],
                       axis=0)
    kernel.last_exec_time_ns = res.exec_time_ns
    kernel.last_results = res
    return z

